# revision 28
# baseline (speedup 1.0000x reference)
"""Trainium2 Bass kernel for EquidistantDiscreteContinuousConv3d.

Math: out = conv3d(x, einsum('ogk,kzyx->ogzyx', weight, psi_local), stride 2,
pad 2) + bias, with x [2,8,128,128,128] -> out [2,16,64,64,64].

KEY STRUCTURE: although the basis nominally spans a 5^3 stencil, the
reference computes r = sqrt(d^2 + 1e-12), which pushes the six radius-2
offsets (+-2,0,0),(0,+-2,0),(0,0,+-2) infinitesimally OUTSIDE r_cutoff, so
psi (and hence the contracted kernel for ANY weights) is identically zero
there. The effective stencil is exactly the 3x3x3 cube (27 taps). This
kernel exploits that: 9 (dy,dx) passes with a 3-tap z-band instead of 13
passes with a 5-tap band.

Sharding: 8 cores = batch(2) x y-quarters(4); each core computes
out[b, :, :, 16gy:16gy+16] from a y-overlapping, zero-padded input slab
spanning the FULL z range. No collectives.

Device mapping: the tensor engine contracts K = (z_local(16) x ic(8)) = 128
partitions, with M = (oz_sub(8, 7 used) x oc(16)) packed into a block-banded
weight matrix (band encodes the 3 dz taps), looped over the 9 (dy, dx) taps
accumulating in PSUM. A 15-plane window supports 7 output planes -> 10
z-windows x 2 y-halves x 9 taps = 180 matmuls of N=512 per core. rhs slices
come from a phase-decomposed (even/odd y and x) view of each window tile.

Input arrives as 10 window tiles (15 z-planes = partitions 0..119; partition
rows 120-127 are zeroed by the first 6 transfers and never rewritten), each
as two non-overlapping half-DMAs (yo rows [0,9) and [9,17)). Output leaves
as bf16 (upcast on host) to halve write traffic.

Raw Bacc pipeline per core (static, fully unrolled; no TileContext):
  ACT : wtile(j<3), A0, wtile(j>=3), B0, A1, B1 DMAs, throttle on tile-0
        completion (a deep ring queue delays its increment visibility and
        with it the first real matmul), B2, then 20 output DMAs
  SP  : waits tile 0, then A2..A9 / B3..B9 interleaved in tile order,
        then end-of-run sem clear
  PE  : 124 N=64 warmups (clock ramp covering the input wake), then 20
        groups x 9 banded matmuls accumulating in psum bank g%8
  DVE : 20 psum->stage bf16 copies (4 rotating stage slots)
"""

import os

import ml_dtypes
import numpy as np

BF16 = ml_dtypes.bfloat16

IC, OC = 8, 16
TAPS_XY = [(dy, dx) for dy in (-1, 0, 1) for dx in (-1, 0, 1)]  # 9 taps
NW = 10  # z-windows of 7 (last: 1) output planes
NG = 2 * NW  # groups: g = 2*w + t, t = y-half of the 16-row output quarter
SUB_FREE = 36 * 132  # window tile free size: (yo 18, yp 2, px 2, xe 66)
ROW = 2 * 2 * 66  # one yo row = (yp, px, xe) block of 264 elements
A_END = 9 * ROW  # half A = yo [0,9): everything group t=0 touches
B_END = 17 * ROW  # half B = yo [9,17); row 17 is never read
NSLOT = 8
N_CORES = 8

_MODULE = None
LAST_RESULT = None  # BassKernelResults of the most recent run (for test harness)


def _oz_per(w):
    return 7 if w < NW - 1 else 1


def _build_module():
    from contextlib import ExitStack

    import concourse.bacc as bacc
    import concourse.mybir as mybir

    f32 = mybir.dt.float32
    bf16 = mybir.dt.bfloat16

    nc = bacc.Bacc()
    x_in = nc.dram_tensor("xc", [NW, 128, SUB_FREE], bf16, kind="ExternalInput")
    w_in = nc.dram_tensor("wc", [128, 9 * 128], bf16, kind="ExternalInput")
    out = nc.dram_tensor("out", [64, 16, 16, 64], bf16, kind="ExternalOutput")

    with ExitStack() as ctx:
        wsem = ctx.enter_context(nc.semaphore("wsem"))
        wsemB = ctx.enter_context(nc.semaphore("wsemB"))
        xsA = [ctx.enter_context(nc.semaphore(f"xsemA{i}")) for i in range(NW)]
        xsB = [ctx.enter_context(nc.semaphore(f"xsemB{i}")) for i in range(NW)]
        pesem = ctx.enter_context(nc.semaphore("pesem"))
        dvsem = ctx.enter_context(nc.semaphore("dvsem"))
        osem = ctx.enter_context(nc.semaphore("osem"))
        wtile = ctx.enter_context(nc.sbuf_tensor("wtile", [128, 9 * 128], bf16))
        xts = [
            ctx.enter_context(nc.sbuf_tensor(f"xt{i}", [128, SUB_FREE], bf16))
            for i in range(NSLOT)
        ]
        stgs = [
            ctx.enter_context(nc.sbuf_tensor(f"stg{i}", [128, 512], bf16))
            for i in range(4)
        ]
        pss = [
            ctx.enter_context(nc.psum_tensor(f"ps{i}", [128, 512], f32))
            for i in range(8)
        ]
        x5s = [
            t[:].rearrange("p (a b d c) -> p a b d c", a=18, b=2, d=2, c=66)
            for t in xts
        ]

        def adma(eng, i):
            # first NSLOT transfers carry host zeros into partition rows
            # 120-127 (never rewritten - the banded weights are zero there, so
            # they must not be NaN garbage); later tiles skip those rows.
            # per-tile semaphores: no same-sem concurrency hazard, no pacing;
            # only slot reuse (i%NSLOT) gates on the PE having drained it
            P = 128 if i < NSLOT else 120
            if i >= NSLOT:
                eng.wait_ge(pesem, 2 * (i - NSLOT) + 2)
            eng.dma_start(
                out=xts[i % NSLOT][:P, 0:A_END],
                in_=x_in[i, 0:P, 0:A_END],
            ).then_inc(xsA[i], 16)

        def bdma(eng, i):
            P = 128 if i < NSLOT else 120
            if i >= NSLOT:
                eng.wait_ge(pesem, 2 * (i - NSLOT) + 2)
            eng.dma_start(
                out=xts[i % NSLOT][:P, A_END:B_END],
                in_=x_in[i, 0:P, A_END:B_END],
            ).then_inc(xsB[i], 16)

        with nc.Block() as block:

            @block.scalar
            def _(act):
                # weight blocks j<3 ride ahead of tile 0's A half; the rest
                # follows - group 0 only needs block j at its j-th matmul, so
                # the first-matmul gate is max(wA, A0) instead of (wtile, A0)
                act.dma_start(
                    out=wtile[:, 0 : 3 * 128], in_=w_in[:, 0 : 3 * 128]
                ).then_inc(wsem, 16)
                adma(act, 0)
                act.dma_start(
                    out=wtile[:, 3 * 128 :], in_=w_in[:, 3 * 128 :]
                ).then_inc(wsemB, 16)
                bdma(act, 0)
                adma(act, 1)
                bdma(act, 1)
                # throttle: keep the ring queue shallow until tile 0 lands
                # (deeper queues delay its completion-increment visibility)
                act.wait_ge(xsA[0], 16)
                bdma(act, 2)

                for s in range(NG):
                    w, t = divmod(s, 2)
                    M = _oz_per(w) * 16
                    act.wait_ge(dvsem, s + 1)
                    dst = out[
                        7 * w : 7 * w + _oz_per(w), :, 8 * t : 8 * t + 8, :
                    ].rearrange("a b c d -> (a b) (c d)")
                    act.dma_start(out=dst, in_=stgs[s % 4][:M, :]).then_inc(osem, 16)

            @block.sync
            def _(sp):
                # hold the main stream until tile 0 lands: early ring flood
                # delays tile 0's completion-increment visibility and with it
                # the first real matmul
                sp.wait_ge(xsA[0], 16)
                # A and B halves interleaved in tile order so the rings
                # deliver tiles in consumption order at full bandwidth
                for i in range(2, NW):
                    adma(sp, i)
                    if i >= 3:
                        bdma(sp, i)
                # re-execution safety: clear sems once everything is done
                sp.wait_ge(osem, 16 * NG)
                for sem in [wsem, wsemB, pesem, dvsem, osem] + xsA + xsB:
                    sp.sem_clear(sem)

            @block.tensor
            def _(pe):
                # warm-up: cheap N=64 throwaway matmuls keep PE busy from the
                # preamble until the first input lands, so the clock gate is
                # ramped for every real matmul. Inputs may be mid-DMA garbage;
                # psum bank 7 is discarded by its first start=True.
                for _ in range(124):
                    pe.matmul(
                        pss[7][:, 0:64], wtile[:, 0:128], wtile[:, 0:64],
                        start=True, stop=True,
                    )
                pe.wait_ge(wsem, 16)
                for g in range(NG):
                    w, t = divmod(g, 2)
                    pe.wait_ge(xsA[w], 16)
                    if t == 1:
                        pe.wait_ge(xsB[w], 16)
                    if g >= 8:
                        pe.wait_ge(dvsem, g - 7)  # psum bank g%8 evacuated
                    x5 = x5s[w % NSLOT]
                    ps = pss[g % 8]
                    for j, (dy, dx) in enumerate(TAPS_XY):
                        if g == 0 and j == 3:
                            pe.wait_ge(wsemB, 16)  # weight blocks j>=3
                        jy, py = divmod(dy + 2, 2)
                        jx, px = divmod(dx + 2, 2)
                        a0 = 8 * t + jy
                        rhs = x5[
                            :, a0 : a0 + 8, py : py + 1, px : px + 1, jx : jx + 64
                        ]
                        mm = pe.matmul(
                            ps[:],
                            wtile[:, j * 128 : (j + 1) * 128],
                            rhs,
                            start=(j == 0),
                            stop=(j == len(TAPS_XY) - 1),
                        )
                        if j == len(TAPS_XY) - 1:
                            mm.then_inc(pesem, 1)

            @block.vector
            def _(dve):
                for g in range(NG):
                    M = _oz_per(g // 2) * 16
                    if g >= 4:
                        dve.wait_ge(osem, 16 * (g - 3))  # stage slot g%4 free
                    dve.wait_ge(pesem, g + 1)
                    dve.tensor_copy(
                        out=stgs[g % 4][:M, :], in_=pss[g % 8][:M]
                    ).then_inc(dvsem, 1)

    nc.compile()
    return nc


def _get_module():
    global _MODULE
    if _MODULE is None:
        _MODULE = _build_module()
    return _MODULE


def _band_weights(w5):
    """wc[k=(zl*8+ic), j*128 + s*16 + oc] block-banded weights.

    Window-local: output plane s (0..6) of any window reads tile-local planes
    zl = 2*s + dzi (dzi = dz+1, dz in {-1,0,1}); rows 120-127 and M-columns
    112-127 stay zero."""
    wc = np.zeros((128, 9, 8, 16), np.float32)
    for j, (dy, dx) in enumerate(TAPS_XY):
        for dzi in range(3):
            blk = w5[:, :, dzi + 1, dy + 2, dx + 2].T  # [ic, oc]
            for s in range(7):
                zl = 2 * s + dzi
                wc[zl * 8 : (zl + 1) * 8, j, s, :] = blk
    return np.ascontiguousarray(wc.reshape(128, 9 * 128))


def _shard_core_input(x, b, gy):
    """Per-core padded input as 10 z-window tiles [128, 36*132]."""
    xp = np.zeros((IC, 142, 36, 132), BF16)
    y_lo = 32 * gy - 2
    src_lo, src_hi = max(0, y_lo), min(128, y_lo + 36)
    xp[:, 2:130, src_lo - y_lo : src_hi - y_lo, 2:130] = x[
        b, :, :, src_lo:src_hi, :
    ]
    tiles = np.zeros((NW, 128, SUB_FREE), BF16)
    for w in range(NW):
        u = xp[:, 14 * w + 1 : 14 * w + 16]  # [ic, zl 15, y 36, x 132]
        # de-interleave phases: free = (yo 18, yp 2, px 2, xe 66)
        u = u.reshape(IC, 15, 36, 66, 2).transpose(0, 1, 2, 4, 3)
        u = u.reshape(IC, 15, 18, 2, 2, 66)
        tiles[w, :120] = u.transpose(1, 0, 2, 3, 4, 5).reshape(120, SUB_FREE)
    return tiles


def kernel(x, weight, bias, psi_local):
    global LAST_RESULT
    from concourse.bass_utils import run_bass_kernel_spmd

    x = np.asarray(x, np.float32)
    weight = np.asarray(weight, np.float32)
    bias = np.asarray(bias, np.float32)
    psi_local = np.asarray(psi_local, np.float32)

    w5 = np.einsum("ogk,kzyx->ogzyx", weight, psi_local).astype(np.float32)
    wc = _band_weights(w5).astype(BF16)

    in_maps = []
    for core in range(N_CORES):
        b, gy = divmod(core, 4)
        in_maps.append({"xc": _shard_core_input(x, b, gy), "wc": wc})

    nc = _get_module()
    trace = bool(int(os.environ.get("KERNEL_TRACE", "0")))
    res = run_bass_kernel_spmd(
        nc, in_maps, core_ids=list(range(N_CORES)), trace=trace
    )
    LAST_RESULT = res

    out = np.empty((2, OC, 64, 64, 64), np.float32)
    for core in range(N_CORES):
        b, gy = divmod(core, 4)
        out[b, :, :, 16 * gy : 16 * gy + 16] = (
            res.results[core]["out"].astype(np.float32).transpose(1, 0, 2, 3)
        )
    out += bias[None, :, None, None, None]
    return out


# revision 32
# speedup vs baseline: 1.0322x; 1.0322x over previous
"""Trainium2 Bass kernel for EquidistantDiscreteContinuousConv3d.

Math: out = conv3d(x, einsum('ogk,kzyx->ogzyx', weight, psi_local), stride 2,
pad 2) + bias, with x [2,8,128,128,128] -> out [2,16,64,64,64].

KEY STRUCTURE: although the basis nominally spans a 5^3 stencil, the
reference computes r = sqrt(d^2 + 1e-12), which pushes the six radius-2
offsets (+-2,0,0),(0,+-2,0),(0,0,+-2) infinitesimally OUTSIDE r_cutoff, so
psi (and hence the contracted kernel for ANY weights) is identically zero
there. The effective stencil is exactly the 3x3x3 cube (27 taps). This
kernel exploits that: 9 (dy,dx) passes with a 3-tap z-band instead of 13
passes with a 5-tap band.

Sharding: 8 cores = batch(2) x y-quarters(4); each core computes
out[b, :, :, 16gy:16gy+16] from a y-overlapping, zero-padded input slab
spanning the FULL z range. No collectives.

Device mapping: the tensor engine contracts K = (z_local(16) x ic(8)) = 128
partitions, with M = (oz_sub(8, 7 used) x oc(16)) packed into a block-banded
weight matrix (band encodes the 3 dz taps), looped over the 9 (dy, dx) taps
accumulating in PSUM. A 15-plane window supports 7 output planes -> 10
z-windows x 2 y-halves x 9 taps = 180 matmuls of N=512 per core. rhs slices
come from a phase-decomposed (even/odd y and x) view of each window tile.

Input arrives as 10 window tiles (15 z-planes = partitions 0..119; partition
rows 120-127 are zeroed by the first 6 transfers and never rewritten), each
as two non-overlapping half-DMAs (yo rows [0,9) and [9,17)). Output leaves
as bf16 (upcast on host) to halve write traffic.

Raw Bacc pipeline per core (static, fully unrolled; no TileContext):
  ACT : wtile(j<3), A0, wtile(j>=3), B0, A1, B1 DMAs, throttle on tile-0
        completion (a deep ring queue delays its increment visibility and
        with it the first real matmul), B2, then 20 output DMAs
  SP  : waits tile 0, then A2..A9 / B3..B9 interleaved in tile order,
        then end-of-run sem clear
  PE  : 124 N=64 warmups (clock ramp covering the input wake), then 20
        groups x 9 banded matmuls accumulating in psum bank g%8
  DVE : 20 psum->stage bf16 copies (4 rotating stage slots)
"""

import os

import ml_dtypes
import numpy as np

BF16 = ml_dtypes.bfloat16

IC, OC = 8, 16
TAPS_XY = [(dy, dx) for dy in (-1, 0, 1) for dx in (-1, 0, 1)]  # 9 taps
NW = 10  # z-windows of 7 (last: 1) output planes
NG = 2 * NW  # groups: g = 2*w + t, t = y-half of the 16-row output quarter
SUB_FREE = 36 * 132  # window tile free size: (yo 18, yp 2, px 2, xe 66)
ROW = 2 * 2 * 66  # one yo row = (yp, px, xe) block of 264 elements
A_END = 9 * ROW  # half A = yo [0,9): everything group t=0 touches
B_END = 17 * ROW  # half B = yo [9,17); row 17 is never read
NSLOT = 8
N_CORES = 8

_MODULE = None
LAST_RESULT = None  # BassKernelResults of the most recent run (for test harness)


def _oz_per(w):
    return 7 if w < NW - 1 else 1


def _build_module():
    from contextlib import ExitStack

    import concourse.bacc as bacc
    import concourse.mybir as mybir

    f32 = mybir.dt.float32
    bf16 = mybir.dt.bfloat16

    nc = bacc.Bacc()
    x_in = nc.dram_tensor("xc", [NW, 128, SUB_FREE], bf16, kind="ExternalInput")
    w_in = nc.dram_tensor("wc", [128, 9 * 128], bf16, kind="ExternalInput")
    out = nc.dram_tensor("out", [64, 16, 16, 64], bf16, kind="ExternalOutput")

    with ExitStack() as ctx:
        wsem = ctx.enter_context(nc.semaphore("wsem"))
        wsemB = ctx.enter_context(nc.semaphore("wsemB"))
        sink = ctx.enter_context(nc.semaphore("sink"))
        xsA = [ctx.enter_context(nc.semaphore(f"xsemA{i}")) for i in range(NW)]
        xsB = [ctx.enter_context(nc.semaphore(f"xsemB{i}")) for i in range(NW)]
        pesem = ctx.enter_context(nc.semaphore("pesem"))
        dvsem = ctx.enter_context(nc.semaphore("dvsem"))
        osem = ctx.enter_context(nc.semaphore("osem"))
        wtile = ctx.enter_context(nc.sbuf_tensor("wtile", [128, 9 * 128], bf16))
        xts = [
            ctx.enter_context(nc.sbuf_tensor(f"xt{i}", [128, SUB_FREE], bf16))
            for i in range(NSLOT)
        ]
        stgs = [
            ctx.enter_context(nc.sbuf_tensor(f"stg{i}", [128, 512], bf16))
            for i in range(4)
        ]
        pss = [
            ctx.enter_context(nc.psum_tensor(f"ps{i}", [128, 512], f32))
            for i in range(8)
        ]
        x5s = [
            t[:].rearrange("p (a b d c) -> p a b d c", a=18, b=2, d=2, c=66)
            for t in xts
        ]

        def adma(eng, i):
            # first NSLOT transfers carry host zeros into partition rows
            # 120-127 (never rewritten - the banded weights are zero there, so
            # they must not be NaN garbage); later tiles skip those rows.
            # per-tile semaphores: no same-sem concurrency hazard, no pacing;
            # only slot reuse (i%NSLOT) gates on the PE having drained it
            P = 128 if i < NSLOT else 120
            if i >= NSLOT:
                eng.wait_ge(pesem, 2 * (i - NSLOT) + 2)
            eng.dma_start(
                out=xts[i % NSLOT][:P, 0:A_END],
                in_=x_in[i, 0:P, 0:A_END],
            ).then_inc(xsA[i], 16)

        def bdma(eng, i):
            P = 128 if i < NSLOT else 120
            if i >= NSLOT:
                eng.wait_ge(pesem, 2 * (i - NSLOT) + 2)
            eng.dma_start(
                out=xts[i % NSLOT][:P, A_END:B_END],
                in_=x_in[i, 0:P, A_END:B_END],
            ).then_inc(xsB[i], 16)

        with nc.Block() as block:

            @block.scalar
            def _(act):
                # weight blocks j<3 ride ahead of tile 0's A half; the rest
                # follows - group 0 only needs block j at its j-th matmul, so
                # the first-matmul gate is max(wA, A0) instead of (wtile, A0)
                act.dma_start(
                    out=wtile[:, 0 : 3 * 128], in_=w_in[:, 0 : 3 * 128]
                ).then_inc(wsem, 16)
                adma(act, 0)
                act.dma_start(
                    out=wtile[:, 3 * 128 :], in_=w_in[:, 3 * 128 :]
                ).then_inc(wsemB, 16)
                bdma(act, 0)
                adma(act, 1)
                bdma(act, 1)
                # throttle: keep the ring queue shallow until tile 0 lands
                # (deeper queues delay its completion-increment visibility)
                act.wait_ge(xsA[0], 16)
                bdma(act, 2)

                for s in range(NG):
                    w, t = divmod(s, 2)
                    M = _oz_per(w) * 16
                    act.wait_ge(dvsem, s + 1)
                    dst = out[
                        7 * w : 7 * w + _oz_per(w), :, 8 * t : 8 * t + 8, :
                    ].rearrange("a b c d -> (a b) (c d)")
                    dma = act.dma_start(out=dst, in_=stgs[s % 4][:M, :])
                    if s < NG - 4:
                        dma.then_inc(osem, 16)
                    else:
                        # the last 4 outputs have no downstream waiter (their
                        # stage slots are never reused); incrementing a sink
                        # semaphore nobody reads lets the end-of-run clear
                        # overlap their flight with the fixed teardown relay
                        dma.then_inc(sink, 16)

            @block.sync
            def _(sp):
                # hold the main stream until tile 0 lands: early ring flood
                # delays tile 0's completion-increment visibility and with it
                # the first real matmul
                sp.wait_ge(xsA[0], 16)
                # A and B halves interleaved in tile order so the rings
                # deliver tiles in consumption order at full bandwidth
                for i in range(2, NW):
                    adma(sp, i)
                    if i >= 3:
                        bdma(sp, i)
                # re-execution safety: clear sems once every increment that
                # will ever fire has landed (tracked odmas + all copies)
                sp.wait_ge(osem, 16 * (NG - 4))
                sp.wait_ge(dvsem, NG)
                for sem in [wsem, wsemB, pesem, dvsem, osem] + xsA + xsB:
                    sp.sem_clear(sem)

            @block.tensor
            def _(pe):
                # warm-up: cheap N=64 throwaway matmuls keep PE busy from the
                # preamble until the first input lands, so the clock gate is
                # ramped for every real matmul. Inputs may be mid-DMA garbage;
                # psum bank 7 is discarded by its first start=True.
                for _ in range(124):
                    pe.matmul(
                        pss[7][:, 0:64], wtile[:, 0:128], wtile[:, 0:64],
                        start=True, stop=True,
                    )
                pe.wait_ge(wsem, 16)
                for g in range(NG):
                    w, t = divmod(g, 2)
                    pe.wait_ge(xsA[w], 16)
                    if t == 1:
                        pe.wait_ge(xsB[w], 16)
                    if g >= 8:
                        pe.wait_ge(dvsem, g - 7)  # psum bank g%8 evacuated
                    x5 = x5s[w % NSLOT]
                    ps = pss[g % 8]
                    for j, (dy, dx) in enumerate(TAPS_XY):
                        if g == 0 and j == 3:
                            pe.wait_ge(wsemB, 16)  # weight blocks j>=3
                        jy, py = divmod(dy + 2, 2)
                        jx, px = divmod(dx + 2, 2)
                        a0 = 8 * t + jy
                        rhs = x5[
                            :, a0 : a0 + 8, py : py + 1, px : px + 1, jx : jx + 64
                        ]
                        mm = pe.matmul(
                            ps[:],
                            wtile[:, j * 128 : (j + 1) * 128],
                            rhs,
                            start=(j == 0),
                            stop=(j == len(TAPS_XY) - 1),
                        )
                        if j == len(TAPS_XY) - 1:
                            mm.then_inc(pesem, 1)

            @block.vector
            def _(dve):
                for g in range(NG):
                    M = _oz_per(g // 2) * 16
                    if g >= 4:
                        dve.wait_ge(osem, 16 * (g - 3))  # stage slot g%4 free
                    dve.wait_ge(pesem, g + 1)
                    dve.tensor_copy(
                        out=stgs[g % 4][:M, :], in_=pss[g % 8][:M]
                    ).then_inc(dvsem, 1)

    nc.compile()
    return nc


def _get_module():
    global _MODULE
    if _MODULE is None:
        _MODULE = _build_module()
    return _MODULE


def _band_weights(w5):
    """wc[k=(zl*8+ic), j*128 + s*16 + oc] block-banded weights.

    Window-local: output plane s (0..6) of any window reads tile-local planes
    zl = 2*s + dzi (dzi = dz+1, dz in {-1,0,1}); rows 120-127 and M-columns
    112-127 stay zero."""
    wc = np.zeros((128, 9, 8, 16), np.float32)
    for j, (dy, dx) in enumerate(TAPS_XY):
        for dzi in range(3):
            blk = w5[:, :, dzi + 1, dy + 2, dx + 2].T  # [ic, oc]
            for s in range(7):
                zl = 2 * s + dzi
                wc[zl * 8 : (zl + 1) * 8, j, s, :] = blk
    return np.ascontiguousarray(wc.reshape(128, 9 * 128))


def _shard_core_input(x, b, gy):
    """Per-core padded input as 10 z-window tiles [128, 36*132]."""
    xp = np.zeros((IC, 142, 36, 132), BF16)
    y_lo = 32 * gy - 2
    src_lo, src_hi = max(0, y_lo), min(128, y_lo + 36)
    xp[:, 2:130, src_lo - y_lo : src_hi - y_lo, 2:130] = x[
        b, :, :, src_lo:src_hi, :
    ]
    tiles = np.zeros((NW, 128, SUB_FREE), BF16)
    for w in range(NW):
        u = xp[:, 14 * w + 1 : 14 * w + 16]  # [ic, zl 15, y 36, x 132]
        # de-interleave phases: free = (yo 18, yp 2, px 2, xe 66)
        u = u.reshape(IC, 15, 36, 66, 2).transpose(0, 1, 2, 4, 3)
        u = u.reshape(IC, 15, 18, 2, 2, 66)
        tiles[w, :120] = u.transpose(1, 0, 2, 3, 4, 5).reshape(120, SUB_FREE)
    return tiles


def kernel(x, weight, bias, psi_local):
    global LAST_RESULT
    from concourse.bass_utils import run_bass_kernel_spmd

    x = np.asarray(x, np.float32)
    weight = np.asarray(weight, np.float32)
    bias = np.asarray(bias, np.float32)
    psi_local = np.asarray(psi_local, np.float32)

    w5 = np.einsum("ogk,kzyx->ogzyx", weight, psi_local).astype(np.float32)
    wc = _band_weights(w5).astype(BF16)

    in_maps = []
    for core in range(N_CORES):
        b, gy = divmod(core, 4)
        in_maps.append({"xc": _shard_core_input(x, b, gy), "wc": wc})

    nc = _get_module()
    trace = bool(int(os.environ.get("KERNEL_TRACE", "0")))
    res = run_bass_kernel_spmd(
        nc, in_maps, core_ids=list(range(N_CORES)), trace=trace
    )
    LAST_RESULT = res

    out = np.empty((2, OC, 64, 64, 64), np.float32)
    for core in range(N_CORES):
        b, gy = divmod(core, 4)
        out[b, :, :, 16 * gy : 16 * gy + 16] = (
            res.results[core]["out"].astype(np.float32).transpose(1, 0, 2, 3)
        )
    out += bias[None, :, None, None, None]
    return out


# revision 36
# speedup vs baseline: 1.0582x; 1.0251x over previous
"""Trainium2 Bass kernel for EquidistantDiscreteContinuousConv3d.

Math: out = conv3d(x, einsum('ogk,kzyx->ogzyx', weight, psi_local), stride 2,
pad 2) + bias, with x [2,8,128,128,128] -> out [2,16,64,64,64].

KEY STRUCTURE: although the basis nominally spans a 5^3 stencil, the
reference computes r = sqrt(d^2 + 1e-12), which pushes the six radius-2
offsets (+-2,0,0),(0,+-2,0),(0,0,+-2) infinitesimally OUTSIDE r_cutoff, so
psi (and hence the contracted kernel for ANY weights) is identically zero
there. The effective stencil is exactly the 3x3x3 cube (27 taps). This
kernel exploits that: 9 (dy,dx) passes with a 3-tap z-band instead of 13
passes with a 5-tap band.

Sharding: 8 cores = batch(2) x y-quarters(4); each core computes
out[b, :, :, 16gy:16gy+16] from a y-overlapping, zero-padded input slab
spanning the FULL z range. No collectives.

Device mapping: the tensor engine contracts K = (z_local(16) x ic(8)) = 128
partitions, with M = (oz_sub(8, 7 used) x oc(16)) packed into a block-banded
weight matrix (band encodes the 3 dz taps), looped over the 9 (dy, dx) taps
accumulating in PSUM. A 15-plane window supports 7 output planes -> 10
z-windows x 2 y-halves x 9 taps = 180 matmuls of N=512 per core. rhs slices
come from a phase-decomposed (even/odd y and x) view of each window tile.

Input arrives as 10 window tiles (15 z-planes = partitions 0..119; partition
rows 120-127 are zeroed by the first 6 transfers and never rewritten), each
as two non-overlapping half-DMAs (yo rows [0,9) and [9,17)). Output leaves
as bf16 (upcast on host) to halve write traffic.

Raw Bacc pipeline per core (static, fully unrolled; no TileContext):
  ACT : wtile(j<3), A0, wtile(j>=3), B0, A1, B1 DMAs, throttle on tile-0
        completion (a deep ring queue delays its increment visibility and
        with it the first real matmul), B2, then 20 output DMAs
  SP  : waits tile 0, then A2..A9 / B3..B9 interleaved in tile order,
        then end-of-run sem clear
  PE  : 124 N=64 warmups (clock ramp covering the input wake), then 20
        groups x 9 banded matmuls accumulating in psum bank g%8
  DVE : 20 psum->stage bf16 copies (4 rotating stage slots)
"""

import os

import ml_dtypes
import numpy as np

BF16 = ml_dtypes.bfloat16

IC, OC = 8, 16
TAPS_XY = [(dy, dx) for dy in (-1, 0, 1) for dx in (-1, 0, 1)]  # 9 taps
NW = 10  # z-windows of 7 (last: 1) output planes
NG = 2 * NW  # groups: g = 2*w + t, t = y-half of the 16-row output quarter
SUB_FREE = 36 * 132  # window tile free size: (yo 18, yp 2, px 2, xe 66)
ROW = 2 * 2 * 66  # one yo row = (yp, px, xe) block of 264 elements
A_END = 9 * ROW  # half A = yo [0,9): everything group t=0 touches
B_END = 17 * ROW  # half B = yo [9,17); row 17 is never read
NSLOT = 8
N_CORES = 8

_MODULE = None
LAST_RESULT = None  # BassKernelResults of the most recent run (for test harness)


def _oz_per(w):
    return 7 if w < NW - 1 else 1


def _build_module():
    from contextlib import ExitStack

    import concourse.bacc as bacc
    import concourse.mybir as mybir

    f32 = mybir.dt.float32
    bf16 = mybir.dt.bfloat16

    nc = bacc.Bacc()
    x_in = nc.dram_tensor("xc", [NW, 128, SUB_FREE], bf16, kind="ExternalInput")
    w_in = nc.dram_tensor("wc", [128, 9 * 128], bf16, kind="ExternalInput")
    out = nc.dram_tensor("out", [64, 16, 16, 64], bf16, kind="ExternalOutput")

    with ExitStack() as ctx:
        wsem = ctx.enter_context(nc.semaphore("wsem"))
        wsemB = ctx.enter_context(nc.semaphore("wsemB"))
        sink = ctx.enter_context(nc.semaphore("sink"))
        xsA = [ctx.enter_context(nc.semaphore(f"xsemA{i}")) for i in range(NW)]
        xsB = [ctx.enter_context(nc.semaphore(f"xsemB{i}")) for i in range(NW)]
        pesem = ctx.enter_context(nc.semaphore("pesem"))
        dvsem = ctx.enter_context(nc.semaphore("dvsem"))
        osem = ctx.enter_context(nc.semaphore("osem"))
        wtile = ctx.enter_context(nc.sbuf_tensor("wtile", [128, 9 * 128], bf16))
        xts = [
            ctx.enter_context(nc.sbuf_tensor(f"xt{i}", [128, SUB_FREE], bf16))
            for i in range(NSLOT)
        ]
        stgs = [
            ctx.enter_context(nc.sbuf_tensor(f"stg{i}", [128, 512], bf16))
            for i in range(4)
        ]
        pss = [
            ctx.enter_context(nc.psum_tensor(f"ps{i}", [128, 512], f32))
            for i in range(8)
        ]
        x5s = [
            t[:].rearrange("p (a b d c) -> p a b d c", a=18, b=2, d=2, c=66)
            for t in xts
        ]

        def adma(eng, i):
            # first NSLOT transfers carry host zeros into partition rows
            # 120-127 (never rewritten - the banded weights are zero there, so
            # they must not be NaN garbage); later tiles skip those rows.
            # per-tile semaphores: no same-sem concurrency hazard, no pacing;
            # only slot reuse (i%NSLOT) gates on the PE having drained it
            P = 128 if i < NSLOT else 120
            if i >= NSLOT:
                eng.wait_ge(pesem, 2 * (i - NSLOT) + 2)
            eng.dma_start(
                out=xts[i % NSLOT][:P, 0:A_END],
                in_=x_in[i, 0:P, 0:A_END],
            ).then_inc(xsA[i], 16)

        def bdma(eng, i):
            P = 128 if i < NSLOT else 120
            if i >= NSLOT:
                eng.wait_ge(pesem, 2 * (i - NSLOT) + 2)
            eng.dma_start(
                out=xts[i % NSLOT][:P, A_END:B_END],
                in_=x_in[i, 0:P, A_END:B_END],
            ).then_inc(xsB[i], 16)

        with nc.Block() as block:

            @block.scalar
            def _(act):
                # weight blocks j<3 ride ahead of tile 0's A half; the rest
                # follows - group 0 only needs block j at its j-th matmul, so
                # the first-matmul gate is max(wA, A0) instead of (wtile, A0)
                act.dma_start(
                    out=wtile[:, 0 : 3 * 128], in_=w_in[:, 0 : 3 * 128]
                ).then_inc(wsem, 16)
                adma(act, 0)
                act.dma_start(
                    out=wtile[:, 3 * 128 :], in_=w_in[:, 3 * 128 :]
                ).then_inc(wsemB, 16)
                bdma(act, 0)
                adma(act, 1)
                bdma(act, 1)
                # throttle: keep the ring queue shallow until tile 0 lands
                # (deeper queues delay its completion-increment visibility)
                act.wait_ge(xsA[0], 16)
                bdma(act, 2)

                for s in range(NG):
                    w, t = divmod(s, 2)
                    M = _oz_per(w) * 16
                    act.wait_ge(dvsem, s + 1)
                    dst = out[
                        7 * w : 7 * w + _oz_per(w), :, 8 * t : 8 * t + 8, :
                    ].rearrange("a b c d -> (a b) (c d)")
                    dma = act.dma_start(out=dst, in_=stgs[s % 4][:M, :])
                    if s < NG - 4:
                        dma.then_inc(osem, 16)
                    else:
                        # the last 4 outputs have no downstream waiter (their
                        # stage slots are never reused); incrementing a sink
                        # semaphore nobody reads lets the end-of-run clear
                        # overlap their flight with the fixed teardown relay
                        dma.then_inc(sink, 16)

            @block.sync
            def _(sp):
                # hold the main stream until tile 0 lands: early ring flood
                # delays tile 0's completion-increment visibility and with it
                # the first real matmul
                sp.wait_ge(xsA[0], 16)
                # A and B halves interleaved in tile order so the rings
                # deliver tiles in consumption order at full bandwidth
                for i in range(2, NW):
                    adma(sp, i)
                    if i >= 3:
                        bdma(sp, i)
                # re-execution safety: clear sems once every increment that
                # will ever fire has landed (tracked odmas + all copies); the
                # x-tile sems are cleared by the PE in parallel (see below)
                sp.wait_ge(osem, 16 * (NG - 4))
                sp.wait_ge(dvsem, NG)
                for sem in [wsem, wsemB, sink, pesem, dvsem, osem]:
                    sp.sem_clear(sem)

            @block.tensor
            def _(pe):
                # warm-up: cheap N=64 throwaway matmuls keep PE busy from the
                # preamble until the first input lands, so the clock gate is
                # ramped for every real matmul. Inputs may be mid-DMA garbage;
                # psum bank 7 is discarded by its first start=True.
                for _ in range(124):
                    pe.matmul(
                        pss[7][:, 0:64], wtile[:, 0:128], wtile[:, 0:64],
                        start=True, stop=True,
                    )
                pe.wait_ge(wsem, 16)
                for g in range(NG):
                    w, t = divmod(g, 2)
                    pe.wait_ge(xsA[w], 16)
                    if t == 1:
                        pe.wait_ge(xsB[w], 16)
                    if g >= 8:
                        pe.wait_ge(dvsem, g - 7)  # psum bank g%8 evacuated
                    x5 = x5s[w % NSLOT]
                    ps = pss[g % 8]
                    for j, (dy, dx) in enumerate(TAPS_XY):
                        if g == 0 and j == 3:
                            pe.wait_ge(wsemB, 16)  # weight blocks j>=3
                        jy, py = divmod(dy + 2, 2)
                        jx, px = divmod(dx + 2, 2)
                        a0 = 8 * t + jy
                        rhs = x5[
                            :, a0 : a0 + 8, py : py + 1, px : px + 1, jx : jx + 64
                        ]
                        mm = pe.matmul(
                            ps[:],
                            wtile[:, j * 128 : (j + 1) * 128],
                            rhs,
                            start=(j == 0),
                            stop=(j == len(TAPS_XY) - 1),
                        )
                        if j == len(TAPS_XY) - 1:
                            mm.then_inc(pesem, 1)
                # the input-tile sems are fully settled once group 19's waits
                # have sampled (all DMA increments landed, no later waiters);
                # clearing them here runs parallel to the copy/output tail and
                # off the SP clear chain that gates the teardown relay
                for sem in xsA + xsB:
                    pe.sem_clear(sem)

            @block.vector
            def _(dve):
                for g in range(NG):
                    M = _oz_per(g // 2) * 16
                    if g >= 4:
                        dve.wait_ge(osem, 16 * (g - 3))  # stage slot g%4 free
                    dve.wait_ge(pesem, g + 1)
                    dve.tensor_copy(
                        out=stgs[g % 4][:M, :], in_=pss[g % 8][:M]
                    ).then_inc(dvsem, 1)

    nc.compile()
    return nc


def _get_module():
    global _MODULE
    if _MODULE is None:
        _MODULE = _build_module()
    return _MODULE


def _band_weights(w5):
    """wc[k=(zl*8+ic), j*128 + s*16 + oc] block-banded weights.

    Window-local: output plane s (0..6) of any window reads tile-local planes
    zl = 2*s + dzi (dzi = dz+1, dz in {-1,0,1}); rows 120-127 and M-columns
    112-127 stay zero."""
    wc = np.zeros((128, 9, 8, 16), np.float32)
    for j, (dy, dx) in enumerate(TAPS_XY):
        for dzi in range(3):
            blk = w5[:, :, dzi + 1, dy + 2, dx + 2].T  # [ic, oc]
            for s in range(7):
                zl = 2 * s + dzi
                wc[zl * 8 : (zl + 1) * 8, j, s, :] = blk
    return np.ascontiguousarray(wc.reshape(128, 9 * 128))


def _shard_core_input(x, b, gy):
    """Per-core padded input as 10 z-window tiles [128, 36*132]."""
    xp = np.zeros((IC, 142, 36, 132), BF16)
    y_lo = 32 * gy - 2
    src_lo, src_hi = max(0, y_lo), min(128, y_lo + 36)
    xp[:, 2:130, src_lo - y_lo : src_hi - y_lo, 2:130] = x[
        b, :, :, src_lo:src_hi, :
    ]
    tiles = np.zeros((NW, 128, SUB_FREE), BF16)
    for w in range(NW):
        u = xp[:, 14 * w + 1 : 14 * w + 16]  # [ic, zl 15, y 36, x 132]
        # de-interleave phases: free = (yo 18, yp 2, px 2, xe 66)
        u = u.reshape(IC, 15, 36, 66, 2).transpose(0, 1, 2, 4, 3)
        u = u.reshape(IC, 15, 18, 2, 2, 66)
        tiles[w, :120] = u.transpose(1, 0, 2, 3, 4, 5).reshape(120, SUB_FREE)
    return tiles


def kernel(x, weight, bias, psi_local):
    global LAST_RESULT
    from concourse.bass_utils import run_bass_kernel_spmd

    x = np.asarray(x, np.float32)
    weight = np.asarray(weight, np.float32)
    bias = np.asarray(bias, np.float32)
    psi_local = np.asarray(psi_local, np.float32)

    w5 = np.einsum("ogk,kzyx->ogzyx", weight, psi_local).astype(np.float32)
    wc = _band_weights(w5).astype(BF16)

    in_maps = []
    for core in range(N_CORES):
        b, gy = divmod(core, 4)
        in_maps.append({"xc": _shard_core_input(x, b, gy), "wc": wc})

    nc = _get_module()
    trace = bool(int(os.environ.get("KERNEL_TRACE", "0")))
    res = run_bass_kernel_spmd(
        nc, in_maps, core_ids=list(range(N_CORES)), trace=trace
    )
    LAST_RESULT = res

    out = np.empty((2, OC, 64, 64, 64), np.float32)
    for core in range(N_CORES):
        b, gy = divmod(core, 4)
        out[b, :, :, 16 * gy : 16 * gy + 16] = (
            res.results[core]["out"].astype(np.float32).transpose(1, 0, 2, 3)
        )
    out += bias[None, :, None, None, None]
    return out
